# revision 30
# baseline (speedup 1.0000x reference)
"""Distributed GQA attention kernel for one TRN2 chip (8 NeuronCores).

nn_Attention: B=2, S=2048, D=2048, H=32 q-heads, KV=8 kv-heads, HD=64,
RoPE (interleaved pairs), causal softmax, GQA repeat 4, output proj.

Sharding (tensor-parallel over heads): core c owns q-heads 4c..4c+3 and
kv-head c; per-head attention outputs are exchanged with an AllToAll (bf16)
so core c computes the wo projection for tokens [256c:256c+256) of each
batch only.

v2 structure:
  * All inputs pre-cast to bf16 AND pre-transposed/tiled on the host: x
    arrives d-major ([128, tb, dt, TB]) so the kernel does no PE transposes
    and no cast DMAs for x; weights/tables arrive pre-tiled bf16.
  * QKV PSUM is single-buffered but evicted immediately to bf16 SBUF
    (ACT in the prologue, DVE when running as fillers) so the next token
    block's matmuls never wait on RoPE; RoPE then runs on bf16 operands in
    DVE 2x mode.  K-RoPE uses an interleaved [cos;sin;cos;sin] staircase
    table to halve instruction count; V is sliced from the same evicted
    kv tile (no separate eviction).
  * Scores run two heads CONCURRENTLY in the PE array via row tiling
    (K=64: head-even rows 0:63, head-odd rows 64:127) into one
    [128,2,512] PSUM tile; exp consumes both heads per ACT instruction.
  * Attention emitted in DESCENDING kt/qt order so expS bufs=1 still
    overlaps the next pair (slot 15 frees after the first PV step).
  * PV output, normalize (recip + per-partition scale on ACT/DVE
    alternately), f32 transpose staging share one PSUM slot; a2a staging
    is one DMA per head (DRAM AP rearranged j-major).
"""
from contextlib import ExitStack

import numpy as np
import ml_dtypes

import concourse.bass as bass
import concourse.mybir as mybir
import concourse.tile as tile
from concourse import bacc
from concourse.bass_utils import run_bass_kernel_spmd
from concourse.masks import make_identity

F32 = mybir.dt.float32
BF16 = mybir.dt.bfloat16
AF = mybir.ActivationFunctionType

NC_CORES = 8
B = 2
S = 2048
D = 2048
H = 32
KV = 8
HD = 64
HPC = H // NC_CORES      # 4 q heads per core
EQ = HPC * HD            # 256
T = B * S
TB = 256                 # phase-1 token block
NTB = T // TB            # 16
KTILES = S // 128
DT = D // 128
TSLICE = T // NC_CORES
BSL = TSLICE // B        # per-batch token slice each core outputs
QCH = 512                # exp / scores q-chunk


def build(reps: int = 1, timeline: bool = False):
    nc = bacc.Bacc("TRN2", target_bir_lowering=False, debug=False,
                   num_devices=NC_CORES)

    xTt = nc.dram_tensor("xTt", [128, NTB, DT, TB], BF16, kind="ExternalInput")
    cos4 = nc.dram_tensor("cos4", [128, S], BF16, kind="ExternalInput")
    sin4 = nc.dram_tensor("sin4", [128, S], BF16, kind="ExternalInput")
    wqTA = nc.dram_tensor("wqTA", [128, DT, 128], BF16, kind="ExternalInput")
    wqTB = nc.dram_tensor("wqTB", [128, DT, 128], BF16, kind="ExternalInput")
    wkvT = nc.dram_tensor("wkvT", [128, DT, 128], BF16, kind="ExternalInput")
    woT = nc.dram_tensor("woT", [D, D], BF16, kind="ExternalInput")
    out = nc.dram_tensor("out", [TSLICE, D], F32, kind="ExternalOutput")

    a2a_in = [[nc.dram_tensor(f"a2a_in{b}{g}", [NC_CORES, 2 * HD, BSL], BF16)
               for g in range(2)] for b in range(B)]
    a2a_out = [[nc.dram_tensor(f"a2a_out{b}{g}", [NC_CORES, 2 * HD, BSL], BF16)
                for g in range(2)] for b in range(B)]
    rg = [list(range(NC_CORES))]

    with tile.TileContext(nc) as tc, ExitStack() as es:
        const = es.enter_context(tc.tile_pool(name="const", bufs=1))
        ident = const.tile([128, 128], BF16, tag="ident")
        make_identity(nc, ident[:])
        identf = const.tile([128, 128], F32, tag="identf")
        make_identity(nc, identf[:])

        qt_pool = es.enter_context(tc.tile_pool(name="qt", bufs=1))
        QTb = [[qt_pool.tile([128, S], BF16, tag=f"QT{b}{g}", name=f"QT{b}{g}")
                for g in range(2)] for b in range(B)]
        KTb = [qt_pool.tile([128, S], BF16, tag=f"KT{b}", name=f"KT{b}")
               for b in range(B)]

        vpool = es.enter_context(tc.tile_pool(name="vaug", bufs=B * KTILES))
        V_aug = []
        for i in range(B * KTILES):
            v = vpool.tile([128, 65], BF16, tag="vaug")
            nc.gpsimd.memset(v[:, 64:65], 1.0)
            V_aug.append(v)

        for _rep in range(reps):
          with tc.tile_pool(name="att", bufs=2) as att, \
               tc.tile_pool(name="expp", bufs=1) as expp, \
               tc.tile_pool(name="psS", bufs=2, space="PSUM") as psSp, \
               tc.tile_pool(name="psOT", bufs=2, space="PSUM") as psOTp:

            # ---------------- emit helpers ----------------
            def p1_chunks(tb, p1sb, xsbp, psQp, psKVp, cos_sb, sin_sb,
                          wq_sb_A, wq_sb_B, wkv_sb, evict_act):
                """Closures emitting phase 1 (load, QKV, evict+RoPE, V) for
                token block tb (TB tokens)."""
                t0 = tb * TB
                bb, c0 = divmod(t0, S)
                state = {}

                def load():
                    xsb = xsbp.tile([128, DT, TB], BF16, tag="xsb",
                                    name=f"xsb{tb}")
                    nc.gpsimd.dma_start(xsb[:], xTt[:, tb, :, :])
                    state["x"] = xsb
                    state["psQ"] = psQp.tile([128, 2, TB], F32, tag="psQ",
                                             name=f"psQ{tb}")
                    state["psKV"] = psKVp.tile([128, TB], F32, tag="psKV",
                                               name=f"psKV{tb}")

                def qkv(which, d0):
                    # One accumulation group at a time per PSUM bank: the two
                    # psQ halves share a bank (2KB zero region), so the A, B
                    # and KV passes each run sequentially over all dt.
                    def f():
                        for dt in range(d0, d0 + 4):
                            st = dict(start=(dt == 0), stop=(dt == DT - 1))
                            x_ = state["x"][:, dt, :]
                            if which == 0:
                                nc.tensor.matmul(state["psQ"][:, 0, :],
                                                 wq_sb_A[:, dt, :], x_, **st)
                            elif which == 1:
                                nc.tensor.matmul(state["psQ"][:, 1, :],
                                                 wq_sb_B[:, dt, :], x_, **st)
                            else:
                                nc.tensor.matmul(state["psKV"][:],
                                                 wkv_sb[:, dt, :], x_, **st)
                    return f

                def rope():
                    q_sb = p1sb.tile([128, 2, TB], BF16, tag="qe",
                                     name=f"qe{tb}")
                    kv_sb = p1sb.tile([128, TB], BF16, tag="kve",
                                      name=f"kve{tb}")
                    if evict_act:
                        nc.scalar.copy(q_sb[:], state["psQ"][:])
                        nc.scalar.copy(kv_sb[:], state["psKV"][:])
                    else:
                        nc.vector.tensor_copy(q_sb[:], state["psQ"][:])
                        nc.vector.tensor_copy(kv_sb[:], state["psKV"][:])
                    qA, qB = q_sb[:, 0, :], q_sb[:, 1, :]
                    cs = cos_sb[:, c0:c0 + TB]
                    sn = sin_sb[:, c0:c0 + TB]
                    # K rope first: it reads psKV (the odd-partition-base
                    # operands must stay in PSUM — the BIR verifier requires
                    # equal SBUF start partitions on TensorTensor inputs) and
                    # releases the psKV ring for the next token block.
                    psKV = state["psKV"]
                    k1 = p1sb.tile([32, TB], BF16, tag="k1", name=f"k1_{tb}")
                    k2 = p1sb.tile([32, TB], BF16, tag="k2", name=f"k2_{tb}")
                    k3 = p1sb.tile([32, TB], BF16, tag="k3", name=f"k3_{tb}")
                    k4 = p1sb.tile([32, TB], BF16, tag="k4", name=f"k4_{tb}")
                    nc.vector.tensor_mul(k1[:], psKV[0:32, :], cs[0:32, :])
                    nc.vector.tensor_mul(k2[:], psKV[32:64, :], sn[0:32, :])
                    nc.vector.tensor_mul(k3[:], psKV[0:32, :], sn[0:32, :])
                    nc.vector.tensor_mul(k4[:], psKV[32:64, :], cs[0:32, :])
                    nc.vector.tensor_sub(KTb[bb][0:32, c0:c0 + TB],
                                         k1[:], k2[:])
                    nc.vector.tensor_add(KTb[bb][32:64, c0:c0 + TB],
                                         k3[:], k4[:])
                    nc.vector.tensor_copy(KTb[bb][64:128, c0:c0 + TB],
                                          KTb[bb][0:64, c0:c0 + TB])
                    t1 = p1sb.tile([128, TB], BF16, tag="t1", name=f"t1_{tb}")
                    t2 = p1sb.tile([128, TB], BF16, tag="t2", name=f"t2_{tb}")
                    t3 = p1sb.tile([128, TB], BF16, tag="t3", name=f"t3_{tb}")
                    t4 = p1sb.tile([128, TB], BF16, tag="t4", name=f"t4_{tb}")
                    nc.vector.tensor_mul(t1[:], qA, cs)
                    nc.vector.tensor_mul(t2[:], qB, sn)
                    nc.vector.tensor_mul(t3[:], qA, sn)
                    nc.vector.tensor_mul(t4[:], qB, cs)
                    Aout = p1sb.tile([128, TB], BF16, tag="Aout", name=f"Ao{tb}")
                    Bout = p1sb.tile([128, TB], BF16, tag="Bout", name=f"Bo{tb}")
                    nc.vector.tensor_sub(Aout[:], t1[:], t2[:])
                    nc.vector.tensor_add(Bout[:], t3[:], t4[:])
                    for h in range(HPC):
                        rb = (h % 2) * 64
                        nc.vector.tensor_copy(
                            QTb[bb][h // 2][rb:rb + 32, c0:c0 + TB],
                            Aout[32 * h:32 * (h + 1), :])
                        nc.vector.tensor_copy(
                            QTb[bb][h // 2][rb + 32:rb + 64, c0:c0 + TB],
                            Bout[32 * h:32 * (h + 1), :])
                    # V: transpose token-major straight from the evicted kv
                    # (psS-tag ring: its readers are always emitted
                    # immediately, unlike the deferred-normalize psO ring)
                    psV = psSp.tile([128, TB], BF16, tag="psS",
                                    name=f"psV{tb}")
                    for i in range(TB // 128):
                        nc.tensor.transpose(psV[:, 64 * i:64 * (i + 1)],
                                            kv_sb[64:128,
                                                  128 * i:128 * (i + 1)],
                                            ident[64:128, 64:128])
                    for i in range(TB // 128):
                        nc.vector.tensor_copy(
                            V_aug[(t0 // 128) + i][:, 0:64],
                            psV[:, 64 * i:64 * (i + 1)])

                return ([load]
                        + [qkv(w, d0) for w in range(3)
                           for d0 in range(0, DT, 4)]
                        + [rope])

            def score_chunk(b, g, expS, kt, s0):
                """Scores (two heads row-tiled) + exp for chunk (kt, s0)."""
                qt_t = QTb[b][g]
                w = min(QCH, S - s0)
                klhs_e = KTb[b][0:64, 128 * kt:128 * (kt + 1)]
                klhs_o = KTb[b][64:128, 128 * kt:128 * (kt + 1)]
                ps = psSp.tile([128, 2, QCH], F32, tag="psS",
                               name=f"psS{b}{g}{kt}_{s0}")
                nc.tensor.matmul(ps[:, 0, 0:w], klhs_e,
                                 qt_t[0:64, s0:s0 + w], start=True, stop=True)
                nc.tensor.matmul(ps[:, 1, 0:w], klhs_o,
                                 qt_t[64:128, s0:s0 + w], start=True, stop=True)
                off = s0 - 128 * kt
                nc.scalar.activation(expS[kt][:, :, off:off + w],
                                     ps[:, :, 0:w], AF.Exp, scale=0.125)
                if s0 == 128 * kt:
                    for j in (0, 1):
                        nc.gpsimd.affine_select(
                            out=expS[kt][:, j, 0:128],
                            in_=expS[kt][:, j, 0:128],
                            compare_op=mybir.AluOpType.is_ge, fill=0.0,
                            base=0, pattern=[[1, 128]], channel_multiplier=-1)

            def alloc_expS(b, g, expS, kt):
                expS[kt] = expp.tile([128, 2, S - 128 * kt], BF16,
                                     tag=f"expS{kt}", name=f"expS{b}{g}{kt}")

            class Stream:
                def __init__(self, closures):
                    self.cl = list(closures)
                    self.i = 0

                def pop(self, n=1):
                    for _ in range(n):
                        if self.i < len(self.cl):
                            self.cl[self.i]()
                            self.i += 1

                def drain(self):
                    self.pop(len(self.cl))

            def make_rows(b, g, expS, work=None, per_chunk=2):
                """Descending score-row closures for pair (2g,2g+1); each
                chunk optionally pulls work-filler closures."""
                rows = []
                for kt in reversed(range(KTILES)):
                    def row(kt=kt):
                        alloc_expS(b, g, expS, kt)
                        for s0 in range(128 * kt, S, QCH):
                            score_chunk(b, g, expS, kt, s0)
                            if work is not None:
                                work.pop(per_chunk)
                    rows.append(row)
                return rows

            def p2_pv(b, g, expS, pv_rows=(), work=None, per_work=2):
                """PV + normalize + a2a staging for pair (2g, 2g+1) of batch
                b.  The two heads' qt2 steps are interleaved so the expS
                slots release in descending order; after each step one score
                row of the NEXT pair (pv_rows, descending) plus some work
                fillers are emitted.  The normalize cluster is deferred by
                one PV step so reciprocal never head-of-line-blocks DVE."""
                ri = 0

                def fill_pv():
                    nonlocal ri
                    if ri < len(pv_rows):
                        pv_rows[ri]()
                        ri += 1
                    if work is not None:
                        work.pop(per_work)

                attnT = {j: att.tile([64, NC_CORES, BSL], BF16, tag="attnT",
                                     name=f"attnT{b}{2 * g + j}")
                         for j in (0, 1)}
                pending = []

                def do_norm(item):
                    j, qt2, psO = item
                    rc = att.tile([128, 2], F32, tag="rc", name=f"rc{j}{qt2}")
                    attn_n = att.tile([128, 2, 64], F32, tag="attn_n",
                                      name=f"an{j}{qt2}")
                    for k in (0, 1):
                        nc.vector.reciprocal(rc[:, k:k + 1],
                                             psO[:, 65 * k + 64:65 * k + 65])
                    for k in (0, 1):
                        # always DVE: an ACT-side scale would head-of-line
                        # block the next pair's exp behind the PV pace
                        nc.vector.tensor_scalar(
                            attn_n[:, k, :],
                            psO[:, 65 * k:65 * k + 64],
                            rc[:, k:k + 1], None,
                            mybir.AluOpType.mult)
                    for k in (0, 1):
                        nc.tensor.transpose(
                            psO[0:64, 130 + 128 * k:130 + 128 * (k + 1)],
                            attn_n[:, k, :], identf[:])
                    nc.vector.tensor_copy(attnT[j][:, qt2, :],
                                          psO[0:64, 130:130 + 256])

                def send(j):
                    dst = a2a_in[b][g].ap()[:, HD * j:HD * (j + 1), :]
                    nc.sync.dma_start(dst.rearrange("j p t -> p j t"),
                                      attnT[j][:])

                for qt2 in reversed(range(KTILES // 2)):
                    for j in (0, 1):
                        while len(pending) >= 2:
                            do_norm(pending.pop(0))
                        # psO cols [0:65) even qt, [65:130) odd qt,
                        # [130:386) f32 transpose staging
                        psO = psOTp.tile([128, TB + 130], F32, tag="psOT",
                                         name=f"psO{b}{g}{j}{qt2}")
                        for k in (1, 0):
                            qt = 2 * qt2 + k
                            for i in range(qt + 1):
                                nc.tensor.matmul(
                                    psO[:, 65 * k:65 * k + 65],
                                    expS[i][:, j,
                                            128 * (qt - i):128 * (qt - i) + 128],
                                    V_aug[b * KTILES + i][:],
                                    start=(i == 0), stop=(i == qt))
                        pending.append((j, qt2, psO))
                    fill_pv()
                while pending:
                    item = pending.pop(0)
                    do_norm(item)
                    if item[1] == 0:
                        send(item[0])
                while ri < len(pv_rows):
                    pv_rows[ri]()
                    ri += 1

            def collective(b, g):
                if timeline:
                    nc.gpsimd.dma_start(a2a_out[b][g][:], a2a_in[b][g][:])
                else:
                    nc.gpsimd.collective_compute(
                        "AllToAll", mybir.AluOpType.bypass, replica_groups=rg,
                        ins=[a2a_in[b][g][:]], outs=[a2a_out[b][g][:]])

            def p3_closures(b, rcvp, p3sb, psWp, wo_sb):
                """Two closure lists: A (pair-g0 half: rcv even-dt, partial
                psW evicted to SBUF) and Bc (pair-g1 half: rcv odd-dt, psW +
                partial -> out).  A only needs collective (b,0)."""
                rcv = {}
                partials = {}

                def loadr(g):
                    def f():
                        for src in range(NC_CORES):
                            dt = 2 * src + g
                            r = rcvp.tile([128, BSL], BF16, tag="rcv",
                                          name=f"rcv{b}_{dt}")
                            nc.sync.dma_start(r[:],
                                              a2a_out[b][g][src, :, :])
                            rcv[dt] = r
                    return f

                def fA(tt, eb):
                    def f():
                        psW = psWp.tile([128, 512], F32, tag="psW",
                                        name=f"psWA{b}{tt}{eb}")
                        for i, dt in enumerate(range(0, DT, 2)):
                            nc.tensor.matmul(
                                psW[:],
                                rcv[dt][:, 128 * tt:128 * (tt + 1)],
                                wo_sb[dt][:, 512 * eb:512 * (eb + 1)],
                                start=(i == 0), stop=(i == DT // 2 - 1))
                        p = p3sb.tile([128, 512], F32, tag="partial", bufs=8,
                                      name=f"pt{b}{tt}{eb}")
                        nc.vector.tensor_copy(p[:], psW[:])
                        partials[(tt, eb)] = p
                    return f

                def fB(tt, eb):
                    def f():
                        psW = psWp.tile([128, 512], F32, tag="psW",
                                        name=f"psWB{b}{tt}{eb}")
                        for i, dt in enumerate(range(1, DT, 2)):
                            nc.tensor.matmul(
                                psW[:],
                                rcv[dt][:, 128 * tt:128 * (tt + 1)],
                                wo_sb[dt][:, 512 * eb:512 * (eb + 1)],
                                start=(i == 0), stop=(i == DT // 2 - 1))
                        osb = p3sb.tile([128, 512], F32, tag="osb",
                                        name=f"osb{b}{tt}{eb}")
                        nc.vector.tensor_add(osb[:], psW[:],
                                             partials[(tt, eb)][:])
                        nc.sync.dma_start(
                            out[b * BSL + 128 * tt:
                                b * BSL + 128 * (tt + 1),
                                512 * eb:512 * (eb + 1)],
                            osb[:])
                    return f

                clsA = [loadr(0)] + [fA(tt, eb)
                                     for tt in range(BSL // 128)
                                     for eb in range(4)]
                clsB = [loadr(1)] + [fB(tt, eb)
                                     for tt in range(BSL // 128)
                                     for eb in range(4)]
                return clsA, clsB

            # ---------------- emission ----------------
            with tc.tile_pool(name="p1c", bufs=1) as p1c, \
                 tc.tile_pool(name="p1sb", bufs=1) as p1sb, \
                 tc.tile_pool(name="xsbp", bufs=2) as xsbp, \
                 tc.tile_pool(name="psQ", bufs=1, space="PSUM") as psQp, \
                 tc.tile_pool(name="psKV", bufs=1, space="PSUM") as psKVp:
                cos_sb = p1c.tile([128, S], BF16, tag="cos")
                sin_sb = p1c.tile([128, S], BF16, tag="sin")
                wq_sb_A = p1c.tile([128, DT, 128], BF16, tag="wqA")
                wq_sb_B = p1c.tile([128, DT, 128], BF16, tag="wqB")
                wkv_sb = p1c.tile([128, DT, 128], BF16, tag="wkv")

                def p1args(evict_act):
                    return (p1sb, xsbp, psQp, psKVp, cos_sb, sin_sb,
                            wq_sb_A, wq_sb_B, wkv_sb, evict_act)

                chunks0 = p1_chunks(0, *p1args(True))
                chunks0[0]()          # tb0 x DMA ahead of table/weight DMAs
                # first dt-quarter of each weight lands first so QKV(tb0)
                # can start ~8us earlier
                for wsb, wdram in ((wq_sb_A, wqTA), (wq_sb_B, wqTB),
                                   (wkv_sb, wkvT)):
                    nc.gpsimd.dma_start(wsb[:, 0:4, :], wdram.ap()[:, 0:4, :])
                for wsb, wdram in ((wq_sb_A, wqTA), (wq_sb_B, wqTB),
                                   (wkv_sb, wkvT)):
                    nc.gpsimd.dma_start(wsb[:, 4:DT, :],
                                        wdram.ap()[:, 4:DT, :])
                nc.gpsimd.dma_start(cos_sb[:], cos4.ap())
                nc.gpsimd.dma_start(sin_sb[:], sin4.ap())

                # merged p1(b0) + progressive ascending scores/exp of pair
                # (0,0): chunk (kt,s0) is enqueued once rope(tb) covering its
                # K row and Q cols has been EMITTED one tb ago (so the PE
                # never head-of-line blocks on an unfinished rope), and
                # popped between the next tb's qkv closures.
                expS00 = [None] * KTILES
                chunk_q = []
                seen = set()

                def enqueue_avail(tb_done):
                    avail_k = 2 * (tb_done + 1)
                    avail_q = TB * (tb_done + 1)
                    for kt in range(avail_k):
                        for s0 in range(128 * kt, S, QCH):
                            w = min(QCH, S - s0)
                            if (kt, s0) in seen or s0 + w > avail_q:
                                continue
                            seen.add((kt, s0))
                            def f(kt=kt, s0=s0):
                                if expS00[kt] is None:
                                    alloc_expS(0, 0, expS00, kt)
                                score_chunk(0, 0, expS00, kt, s0)
                            chunk_q.append(f)

                for c in chunks0[1:]:
                    c()
                enqueue_avail(0)
                for tb in range(1, NTB // 2):
                    cs = p1_chunks(tb, *p1args(True))
                    cs[0]()
                    for c in cs[1:]:
                        c()
                        for _ in range(2):
                            if chunk_q:
                                chunk_q.pop(0)()
                    enqueue_avail(tb)
                while chunk_q:
                    chunk_q.pop(0)()

                W1 = Stream([])
                for tb in range(NTB // 2, NTB):
                    W1.cl.extend(p1_chunks(tb, *p1args(False)))

                # pipeline: each PV phase hosts the next pair's first 8
                # score rows (expS slots free in exactly that order)
                expS01 = [None] * KTILES
                rows01 = make_rows(0, 1, expS01, work=W1, per_chunk=2)
                p2_pv(0, 0, expS00, pv_rows=rows01[0:8], work=W1, per_work=2)
                collective(0, 0)
                for r in rows01[8:]:
                    r()
                W1.drain()

            # p1 pools closed; open wo/p3 pools (SBUF reuse)
            with tc.tile_pool(name="wo", bufs=DT) as wo_pool, \
                 tc.tile_pool(name="p3sb", bufs=2) as p3sb, \
                 tc.tile_pool(name="rcv", bufs=2 * DT) as rcvp, \
                 tc.tile_pool(name="psW", bufs=2, space="PSUM") as psWp:
                wo_sb = []
                for dt in range(DT):
                    w = wo_pool.tile([128, D], BF16, tag="wo", name=f"wo{dt}")
                    nc.gpsimd.dma_start(w[:], woT[128 * dt:128 * (dt + 1), :])
                    wo_sb.append(w)
                A0, B0 = p3_closures(0, rcvp, p3sb, psWp, wo_sb)
                A1, B1 = p3_closures(1, rcvp, p3sb, psWp, wo_sb)

                expS10 = [None] * KTILES
                rows10 = make_rows(1, 0, expS10)
                p2_pv(0, 1, expS01, pv_rows=rows10[0:8])
                collective(0, 1)
                A0[0]()              # rcv even-dt loads (wait coll 0,0)
                for r in rows10[8:]:
                    r()

                expS11 = [None] * KTILES
                rows11 = make_rows(1, 1, expS11)
                WA = Stream(A0[1:])
                p2_pv(1, 0, expS10, pv_rows=rows11[0:8], work=WA, per_work=1)
                collective(1, 0)
                B0[0]()              # rcv odd-dt loads (wait coll 0,1)
                WA.drain()
                for r in rows11[8:]:
                    r()

                A1[0]()              # rcv even-dt b1 (wait coll 1,0)
                WB = Stream(B0[1:] + A1[1:])
                p2_pv(1, 1, expS11, work=WB, per_work=3)
                collective(1, 1)
                WB.drain()
                for c in B1:
                    c()

    nc.compile()
    return nc


def host_inputs(x, freqs_cos, freqs_sin, wq, wk, wv, wo):
    bf16 = ml_dtypes.bfloat16
    x2d = np.asarray(x, dtype=np.float32).reshape(T, D)
    # [128, NTB, DT, TB]: element [p, tb, dt, t] = x2d[TB*tb + t, 128*dt + p]
    xTt = np.ascontiguousarray(
        x2d.reshape(NTB, TB, DT, 128).transpose(3, 0, 2, 1)).astype(bf16)
    fcT = np.asarray(freqs_cos).T.astype(np.float32)   # [32, S]
    fsT = np.asarray(freqs_sin).T.astype(np.float32)
    cos4 = np.ascontiguousarray(np.tile(fcT, (4, 1))).astype(bf16)
    sin4 = np.ascontiguousarray(np.tile(fsT, (4, 1))).astype(bf16)
    woT = np.ascontiguousarray(np.asarray(wo).T).astype(bf16)
    wq = np.asarray(wq)
    wk = np.asarray(wk)
    wv = np.asarray(wv)

    permA = [h * HD + 2 * j for h in range(HPC) for j in range(HD // 2)]
    permB = [h * HD + 2 * j + 1 for h in range(HPC) for j in range(HD // 2)]
    permK = list(range(0, HD, 2)) + list(range(1, HD, 2))

    def tile_weight(w2d):
        # [out 128, D] -> [128 p, DT, 128 out]: [p, dt, e] = w2d[e, 128dt+p]
        return np.ascontiguousarray(
            w2d.T.reshape(DT, 128, 128).transpose(1, 0, 2)).astype(bf16)

    in_maps = []
    for c in range(NC_CORES):
        wq_c = wq[EQ * c: EQ * (c + 1)]
        wk_c = wk[HD * c: HD * (c + 1)]
        wv_c = wv[HD * c: HD * (c + 1)]
        wkv_c = np.concatenate([wk_c[permK], wv_c], axis=0)
        in_maps.append({
            "xTt": xTt, "cos4": cos4, "sin4": sin4,
            "wqTA": tile_weight(wq_c[permA]),
            "wqTB": tile_weight(wq_c[permB]),
            "wkvT": tile_weight(wkv_c),
            "woT": woT,
        })
    return in_maps


def host_gather(results):
    full = np.zeros((B, S, D), np.float32)
    for c in range(NC_CORES):
        o = results[c]["out"]
        for b in range(B):
            full[b, BSL * c: BSL * (c + 1), :] = o[b * BSL:(b + 1) * BSL]
    return full


_NC_CACHE = None


def _get_nc():
    global _NC_CACHE
    if _NC_CACHE is None:
        _NC_CACHE = build()
    return _NC_CACHE


def kernel(x, freqs_cos, freqs_sin, wq, wk, wv, wo):
    nc = _get_nc()
    in_maps = host_inputs(x, freqs_cos, freqs_sin, wq, wk, wv, wo)
    res = run_bass_kernel_spmd(nc, in_maps, core_ids=list(range(NC_CORES)))
    return host_gather(res.results)


# revision 34
# speedup vs baseline: 1.1171x; 1.1171x over previous
"""Distributed GQA attention kernel for one TRN2 chip (8 NeuronCores).

nn_Attention: B=2, S=2048, D=2048, H=32 q-heads, KV=8 kv-heads, HD=64,
RoPE (interleaved pairs), causal softmax, GQA repeat 4, output proj.

Sharding (tensor-parallel over heads): core c owns q-heads 4c..4c+3 and
kv-head c; per-head attention outputs are exchanged with an AllToAll (bf16)
so core c computes the wo projection for tokens [256c:256c+256) of each
batch only.

v2 structure:
  * All inputs pre-cast to bf16 AND pre-transposed/tiled on the host: x
    arrives d-major ([128, tb, dt, TB]) so the kernel does no PE transposes
    and no cast DMAs for x; weights/tables arrive pre-tiled bf16.
  * QKV PSUM is single-buffered but evicted immediately to bf16 SBUF
    (ACT in the prologue, DVE when running as fillers) so the next token
    block's matmuls never wait on RoPE; Q-RoPE then runs on bf16 operands
    in DVE 2x mode; V is sliced from the evicted kv tile (no separate
    eviction).  The Q/KV/K accumulation groups run sequentially because
    each PSUM bank (2KB zero region) tolerates only one open group.
  * Scores run two heads CONCURRENTLY in the PE array via row tiling
    (K=64: head-even rows 0:63, head-odd rows 64:127) into one
    [128,2,512] PSUM tile; exp consumes both heads per ACT instruction.
  * Attention emitted in DESCENDING kt/qt order so expS bufs=1 still
    overlaps the next pair, and the whole schedule is software-pipelined:
    batch-0 pair-0 scores/exp start ASCENDING inside the b0 QKV loop, each
    PV phase hosts the next pair's first 8 score rows (expS slots free in
    exactly that order) plus PE filler closures (b1 QKV / wo halves), and
    each batch's AllToAll is split per head pair with the wo projection
    split into even/odd-dt halves (partial kept in SBUF) so only the last
    collective + half of p3(b1) remain in the tail.
  * PV output, normalize (recip + per-partition scale on ACT/DVE
    alternately), f32 transpose staging share one PSUM slot; a2a staging
    is one DMA per head (DRAM AP rearranged j-major).
"""
from contextlib import ExitStack

import numpy as np
import ml_dtypes

import concourse.bass as bass
import concourse.mybir as mybir
import concourse.tile as tile
from concourse import bacc
from concourse.bass_utils import run_bass_kernel_spmd
from concourse.masks import make_identity

F32 = mybir.dt.float32
BF16 = mybir.dt.bfloat16
F8 = mybir.dt.float8e4
EXP_BIAS = -3.5
AF = mybir.ActivationFunctionType

NC_CORES = 8
B = 2
S = 2048
D = 2048
H = 32
KV = 8
HD = 64
HPC = H // NC_CORES      # 4 q heads per core
EQ = HPC * HD            # 256
T = B * S
TB = 256                 # phase-1 token block
NTB = T // TB            # 16
KTILES = S // 128
DT = D // 128
TSLICE = T // NC_CORES
BSL = TSLICE // B        # per-batch token slice each core outputs
QCH = 512                # exp / scores q-chunk


def build(reps: int = 1, timeline: bool = False):
    nc = bacc.Bacc("TRN2", target_bir_lowering=False, debug=False,
                   num_devices=NC_CORES)

    xTt = nc.dram_tensor("xTt", [128, NTB, DT, TB], BF16, kind="ExternalInput")
    cos4 = nc.dram_tensor("cos4", [128, S], BF16, kind="ExternalInput")
    sin4 = nc.dram_tensor("sin4", [128, S], BF16, kind="ExternalInput")
    wqTA = nc.dram_tensor("wqTA", [128, DT, 128], BF16, kind="ExternalInput")
    wqTB = nc.dram_tensor("wqTB", [128, DT, 128], BF16, kind="ExternalInput")
    wkvT = nc.dram_tensor("wkvT", [128, DT, 128], BF16, kind="ExternalInput")
    woT = nc.dram_tensor("woT", [D, D], BF16, kind="ExternalInput")
    out = nc.dram_tensor("out", [TSLICE, D], F32, kind="ExternalOutput")

    a2a_in = [[nc.dram_tensor(f"a2a_in{b}{g}", [NC_CORES, 2 * HD, BSL], BF16)
               for g in range(2)] for b in range(B)]
    a2a_out = [[nc.dram_tensor(f"a2a_out{b}{g}", [NC_CORES, 2 * HD, BSL], BF16)
                for g in range(2)] for b in range(B)]
    rg = [list(range(NC_CORES))]

    with tile.TileContext(nc) as tc, ExitStack() as es:
        const = es.enter_context(tc.tile_pool(name="const", bufs=1))
        ident = const.tile([128, 128], BF16, tag="ident")
        make_identity(nc, ident[:])
        identf = const.tile([128, 128], F32, tag="identf")
        make_identity(nc, identf[:])

        qt_pool = es.enter_context(tc.tile_pool(name="qt", bufs=1))
        QTb = [[qt_pool.tile([128, S], BF16, tag=f"QT{b}{g}", name=f"QT{b}{g}")
                for g in range(2)] for b in range(B)]
        KTb = [qt_pool.tile([128, S], BF16, tag=f"KT{b}", name=f"KT{b}")
               for b in range(B)]

        vpool = es.enter_context(tc.tile_pool(name="vaug", bufs=B * KTILES))
        V_aug = []
        for i in range(B * KTILES):
            v = vpool.tile([128, 65], BF16, tag="vaug")
            nc.gpsimd.memset(v[:, 64:65], 1.0)
            V_aug.append(v)

        for _rep in range(reps):
          with tc.tile_pool(name="att", bufs=2) as att, \
               tc.tile_pool(name="expp", bufs=1) as expp, \
               tc.tile_pool(name="psS", bufs=2, space="PSUM") as psSp, \
               tc.tile_pool(name="psOT", bufs=2, space="PSUM") as psOTp:

            # ---------------- emit helpers ----------------
            def p1_chunks(tb, p1sb, xsbp, psQp, psKVp, cos_sb, sin_sb,
                          wq_sb_A, wq_sb_B, wkv_sb, evict_act):
                """Closures emitting phase 1 (load, QKV, evict+RoPE, V) for
                token block tb (TB tokens)."""
                t0 = tb * TB
                bb, c0 = divmod(t0, S)
                state = {}

                def load():
                    xsb = xsbp.tile([128, DT, TB], BF16, tag="xsb",
                                    name=f"xsb{tb}")
                    nc.gpsimd.dma_start(xsb[:], xTt[:, tb, :, :])
                    state["x"] = xsb
                    state["psQ"] = psQp.tile([128, 2, TB], F32, tag="psQ",
                                             name=f"psQ{tb}")
                    state["psKV"] = psKVp.tile([128, TB], F32, tag="psKV",
                                               name=f"psKV{tb}")

                def qkv(which, d0):
                    # One accumulation group at a time per PSUM bank: the two
                    # psQ halves share a bank (2KB zero region), so the A, B
                    # and KV passes each run sequentially over all dt.
                    def f():
                        for dt in range(d0, d0 + 4):
                            st = dict(start=(dt == 0), stop=(dt == DT - 1))
                            x_ = state["x"][:, dt, :]
                            if which == 0:
                                nc.tensor.matmul(state["psQ"][:, 0, :],
                                                 wq_sb_A[:, dt, :], x_, **st)
                            elif which == 1:
                                nc.tensor.matmul(state["psQ"][:, 1, :],
                                                 wq_sb_B[:, dt, :], x_, **st)
                            else:
                                nc.tensor.matmul(state["psKV"][:],
                                                 wkv_sb[:, dt, :], x_, **st)
                    return f

                def rope():
                    q_sb = p1sb.tile([128, 2, TB], BF16, tag="qe",
                                     name=f"qe{tb}")
                    kv_sb = p1sb.tile([128, TB], BF16, tag="kve",
                                      name=f"kve{tb}")
                    if evict_act:
                        nc.scalar.copy(q_sb[:], state["psQ"][:])
                        nc.scalar.copy(kv_sb[:], state["psKV"][:])
                    else:
                        nc.vector.tensor_copy(q_sb[:], state["psQ"][:])
                        nc.vector.tensor_copy(kv_sb[:], state["psKV"][:])
                    qA, qB = q_sb[:, 0, :], q_sb[:, 1, :]
                    cs = cos_sb[:, c0:c0 + TB]
                    sn = sin_sb[:, c0:c0 + TB]
                    # K rope first: it reads psKV (the odd-partition-base
                    # operands must stay in PSUM — the BIR verifier requires
                    # equal SBUF start partitions on TensorTensor inputs) and
                    # releases the psKV ring for the next token block.
                    psKV = state["psKV"]
                    k1 = p1sb.tile([32, TB], BF16, tag="k1", name=f"k1_{tb}")
                    k2 = p1sb.tile([32, TB], BF16, tag="k2", name=f"k2_{tb}")
                    k3 = p1sb.tile([32, TB], BF16, tag="k3", name=f"k3_{tb}")
                    k4 = p1sb.tile([32, TB], BF16, tag="k4", name=f"k4_{tb}")
                    nc.vector.tensor_mul(k1[:], psKV[0:32, :], cs[0:32, :])
                    nc.vector.tensor_mul(k2[:], psKV[32:64, :], sn[0:32, :])
                    nc.vector.tensor_mul(k3[:], psKV[0:32, :], sn[0:32, :])
                    nc.vector.tensor_mul(k4[:], psKV[32:64, :], cs[0:32, :])
                    nc.vector.tensor_sub(KTb[bb][0:32, c0:c0 + TB],
                                         k1[:], k2[:])
                    nc.vector.tensor_add(KTb[bb][32:64, c0:c0 + TB],
                                         k3[:], k4[:])
                    nc.vector.tensor_copy(KTb[bb][64:128, c0:c0 + TB],
                                          KTb[bb][0:64, c0:c0 + TB])
                    t1 = p1sb.tile([128, TB], BF16, tag="t1", name=f"t1_{tb}")
                    t2 = p1sb.tile([128, TB], BF16, tag="t2", name=f"t2_{tb}")
                    t3 = p1sb.tile([128, TB], BF16, tag="t3", name=f"t3_{tb}")
                    t4 = p1sb.tile([128, TB], BF16, tag="t4", name=f"t4_{tb}")
                    nc.vector.tensor_mul(t1[:], qA, cs)
                    nc.vector.tensor_mul(t2[:], qB, sn)
                    nc.vector.tensor_mul(t3[:], qA, sn)
                    nc.vector.tensor_mul(t4[:], qB, cs)
                    Aout = p1sb.tile([128, TB], BF16, tag="Aout", name=f"Ao{tb}")
                    Bout = p1sb.tile([128, TB], BF16, tag="Bout", name=f"Bo{tb}")
                    nc.vector.tensor_sub(Aout[:], t1[:], t2[:])
                    nc.vector.tensor_add(Bout[:], t3[:], t4[:])
                    for h in range(HPC):
                        rb = (h % 2) * 64
                        nc.vector.tensor_copy(
                            QTb[bb][h // 2][rb:rb + 32, c0:c0 + TB],
                            Aout[32 * h:32 * (h + 1), :])
                        nc.vector.tensor_copy(
                            QTb[bb][h // 2][rb + 32:rb + 64, c0:c0 + TB],
                            Bout[32 * h:32 * (h + 1), :])
                    # V: transpose token-major straight from the evicted kv
                    # (psS-tag ring: its readers are always emitted
                    # immediately, unlike the deferred-normalize psO ring)
                    psV = psSp.tile([128, TB], BF16, tag="psS",
                                    name=f"psV{tb}")
                    for i in range(TB // 128):
                        nc.tensor.transpose(psV[:, 64 * i:64 * (i + 1)],
                                            kv_sb[64:128,
                                                  128 * i:128 * (i + 1)],
                                            ident[64:128, 64:128])
                    for i in range(TB // 128):
                        nc.vector.tensor_copy(
                            V_aug[(t0 // 128) + i][:, 0:64],
                            psV[:, 64 * i:64 * (i + 1)])

                return ([load]
                        + [qkv(w, d0) for w in range(3)
                           for d0 in range(0, DT, 4)]
                        + [rope])

            def score_chunk(b, g, expS, kt, s0):
                """Scores (two heads row-tiled) + exp for chunk (kt, s0)."""
                qt_t = QTb[b][g]
                w = min(QCH, S - s0)
                klhs_e = KTb[b][0:64, 128 * kt:128 * (kt + 1)]
                klhs_o = KTb[b][64:128, 128 * kt:128 * (kt + 1)]
                ps = psSp.tile([128, 2, QCH], F32, tag="psS",
                               name=f"psS{b}{g}{kt}_{s0}")
                nc.tensor.matmul(ps[:, 0, 0:w], klhs_e,
                                 qt_t[0:64, s0:s0 + w], start=True, stop=True)
                nc.tensor.matmul(ps[:, 1, 0:w], klhs_o,
                                 qt_t[64:128, s0:s0 + w], start=True, stop=True)
                off = s0 - 128 * kt
                nc.scalar.activation(expS[kt][:, :, off:off + w],
                                     ps[:, :, 0:w], AF.Exp, scale=0.125)
                if s0 == 128 * kt:
                    for j in (0, 1):
                        nc.gpsimd.affine_select(
                            out=expS[kt][:, j, 0:128],
                            in_=expS[kt][:, j, 0:128],
                            compare_op=mybir.AluOpType.is_ge, fill=0.0,
                            base=0, pattern=[[1, 128]], channel_multiplier=-1)

            def alloc_expS(b, g, expS, kt):
                expS[kt] = expp.tile([128, 2, S - 128 * kt], BF16,
                                     tag=f"expS{kt}", name=f"expS{b}{g}{kt}")

            class Stream:
                def __init__(self, closures):
                    self.cl = list(closures)
                    self.i = 0

                def pop(self, n=1):
                    for _ in range(n):
                        if self.i < len(self.cl):
                            self.cl[self.i]()
                            self.i += 1

                def drain(self):
                    self.pop(len(self.cl))

            def make_rows(b, g, expS, work=None, per_chunk=2):
                """Descending score-row closures for pair (2g,2g+1); each
                chunk optionally pulls work-filler closures."""
                rows = []
                for kt in reversed(range(KTILES)):
                    def row(kt=kt):
                        alloc_expS(b, g, expS, kt)
                        for s0 in range(128 * kt, S, QCH):
                            score_chunk(b, g, expS, kt, s0)
                            if work is not None:
                                work.pop(per_chunk)
                    rows.append(row)
                return rows

            def p2_pv(b, g, expS, pv_rows=(), work=None, per_work=2):
                """PV + normalize + a2a staging for pair (2g, 2g+1) of batch
                b.  The two heads' qt2 steps are interleaved so the expS
                slots release in descending order; after each step one score
                row of the NEXT pair (pv_rows, descending) plus some work
                fillers are emitted.  The normalize cluster is deferred by
                one PV step so reciprocal never head-of-line-blocks DVE."""
                ri = 0

                def fill_pv():
                    nonlocal ri
                    if ri < len(pv_rows):
                        pv_rows[ri]()
                        ri += 1
                    if work is not None:
                        work.pop(per_work)

                attnT = {j: att.tile([64, NC_CORES, BSL], BF16, tag="attnT",
                                     name=f"attnT{b}{2 * g + j}")
                         for j in (0, 1)}
                pending = []

                def do_norm(item):
                    j, qt2, psO = item
                    rc = att.tile([128, 2], F32, tag="rc", name=f"rc{j}{qt2}")
                    attn_n = att.tile([128, 2, 64], F32, tag="attn_n",
                                      name=f"an{j}{qt2}")
                    for k in (0, 1):
                        nc.vector.reciprocal(rc[:, k:k + 1],
                                             psO[:, 65 * k + 64:65 * k + 65])
                    for k in (0, 1):
                        # always DVE: an ACT-side scale would head-of-line
                        # block the next pair's exp behind the PV pace
                        nc.vector.tensor_scalar(
                            attn_n[:, k, :],
                            psO[:, 65 * k:65 * k + 64],
                            rc[:, k:k + 1], None,
                            mybir.AluOpType.mult)
                    for k in (0, 1):
                        nc.tensor.transpose(
                            psO[0:64, 130 + 128 * k:130 + 128 * (k + 1)],
                            attn_n[:, k, :], identf[:])
                    nc.vector.tensor_copy(attnT[j][:, qt2, :],
                                          psO[0:64, 130:130 + 256])

                def send(j):
                    dst = a2a_in[b][g].ap()[:, HD * j:HD * (j + 1), :]
                    nc.sync.dma_start(dst.rearrange("j p t -> p j t"),
                                      attnT[j][:])

                for qt2 in reversed(range(KTILES // 2)):
                    for j in (0, 1):
                        while len(pending) >= 2:
                            do_norm(pending.pop(0))
                        # psO cols [0:65) even qt, [65:130) odd qt,
                        # [130:386) f32 transpose staging
                        psO = psOTp.tile([128, TB + 130], F32, tag="psOT",
                                         name=f"psO{b}{g}{j}{qt2}")
                        for k in (1, 0):
                            qt = 2 * qt2 + k
                            for i in range(qt + 1):
                                nc.tensor.matmul(
                                    psO[:, 65 * k:65 * k + 65],
                                    expS[i][:, j,
                                            128 * (qt - i):128 * (qt - i) + 128],
                                    V_aug[b * KTILES + i][:],
                                    start=(i == 0), stop=(i == qt))
                        pending.append((j, qt2, psO))
                    fill_pv()
                while pending:
                    item = pending.pop(0)
                    do_norm(item)
                    if item[1] == 0:
                        send(item[0])
                while ri < len(pv_rows):
                    pv_rows[ri]()
                    ri += 1

            def collective(b, g):
                if timeline:
                    nc.gpsimd.dma_start(a2a_out[b][g][:], a2a_in[b][g][:])
                else:
                    nc.gpsimd.collective_compute(
                        "AllToAll", mybir.AluOpType.bypass, replica_groups=rg,
                        ins=[a2a_in[b][g][:]], outs=[a2a_out[b][g][:]])

            def p3_closures(b, rcvp, p3sb, psWp, wo_sb):
                """Two closure lists: A (pair-g0 half: rcv even-dt, partial
                psW evicted to SBUF) and Bc (pair-g1 half: rcv odd-dt, psW +
                partial -> out).  A only needs collective (b,0)."""
                rcv = {}
                partials = {}

                def loadr(g):
                    def f():
                        for src in range(NC_CORES):
                            dt = 2 * src + g
                            r = rcvp.tile([128, BSL], BF16, tag="rcv",
                                          name=f"rcv{b}_{dt}")
                            nc.sync.dma_start(r[:],
                                              a2a_out[b][g][src, :, :])
                            rcv[dt] = r
                    return f

                def fA(tt, eb):
                    def f():
                        psW = psWp.tile([128, 512], F32, tag="psW",
                                        name=f"psWA{b}{tt}{eb}")
                        for i, dt in enumerate(range(0, DT, 2)):
                            nc.tensor.matmul(
                                psW[:],
                                rcv[dt][:, 128 * tt:128 * (tt + 1)],
                                wo_sb[dt][:, 512 * eb:512 * (eb + 1)],
                                start=(i == 0), stop=(i == DT // 2 - 1))
                        p = p3sb.tile([128, 512], F32, tag="partial", bufs=8,
                                      name=f"pt{b}{tt}{eb}")
                        nc.vector.tensor_copy(p[:], psW[:])
                        partials[(tt, eb)] = p
                    return f

                def fB(tt, eb):
                    def f():
                        psW = psWp.tile([128, 512], F32, tag="psW",
                                        name=f"psWB{b}{tt}{eb}")
                        for i, dt in enumerate(range(1, DT, 2)):
                            nc.tensor.matmul(
                                psW[:],
                                rcv[dt][:, 128 * tt:128 * (tt + 1)],
                                wo_sb[dt][:, 512 * eb:512 * (eb + 1)],
                                start=(i == 0), stop=(i == DT // 2 - 1))
                        osb = p3sb.tile([128, 512], F32, tag="osb",
                                        name=f"osb{b}{tt}{eb}")
                        nc.vector.tensor_add(osb[:], psW[:],
                                             partials[(tt, eb)][:])
                        nc.sync.dma_start(
                            out[b * BSL + 128 * tt:
                                b * BSL + 128 * (tt + 1),
                                512 * eb:512 * (eb + 1)],
                            osb[:])
                    return f

                clsA = [loadr(0)] + [fA(tt, eb)
                                     for tt in range(BSL // 128)
                                     for eb in range(4)]
                clsB = [loadr(1)] + [fB(tt, eb)
                                     for tt in range(BSL // 128)
                                     for eb in range(4)]
                return clsA, clsB

            # ---------------- emission ----------------
            with tc.tile_pool(name="p1c", bufs=1) as p1c, \
                 tc.tile_pool(name="p1sb", bufs=1) as p1sb, \
                 tc.tile_pool(name="xsbp", bufs=2) as xsbp, \
                 tc.tile_pool(name="psQ", bufs=1, space="PSUM") as psQp, \
                 tc.tile_pool(name="psKV", bufs=1, space="PSUM") as psKVp:
                cos_sb = p1c.tile([128, S], BF16, tag="cos")
                sin_sb = p1c.tile([128, S], BF16, tag="sin")
                wq_sb_A = p1c.tile([128, DT, 128], BF16, tag="wqA")
                wq_sb_B = p1c.tile([128, DT, 128], BF16, tag="wqB")
                wkv_sb = p1c.tile([128, DT, 128], BF16, tag="wkv")

                def p1args(evict_act):
                    return (p1sb, xsbp, psQp, psKVp, cos_sb, sin_sb,
                            wq_sb_A, wq_sb_B, wkv_sb, evict_act)

                chunks0 = p1_chunks(0, *p1args(True))
                chunks0[0]()          # tb0 x DMA ahead of table/weight DMAs
                # first dt-quarter of each weight lands first so QKV(tb0)
                # can start ~8us earlier
                for wsb, wdram in ((wq_sb_A, wqTA), (wq_sb_B, wqTB),
                                   (wkv_sb, wkvT)):
                    nc.gpsimd.dma_start(wsb[:, 0:4, :], wdram.ap()[:, 0:4, :])
                for wsb, wdram in ((wq_sb_A, wqTA), (wq_sb_B, wqTB),
                                   (wkv_sb, wkvT)):
                    nc.gpsimd.dma_start(wsb[:, 4:DT, :],
                                        wdram.ap()[:, 4:DT, :])
                nc.gpsimd.dma_start(cos_sb[:], cos4.ap())
                nc.gpsimd.dma_start(sin_sb[:], sin4.ap())

                # merged p1(b0) + progressive ascending scores/exp of pair
                # (0,0): chunk (kt,s0) is enqueued once rope(tb) covering its
                # K row and Q cols has been EMITTED one tb ago (so the PE
                # never head-of-line blocks on an unfinished rope), and
                # popped between the next tb's qkv closures.
                expS00 = [None] * KTILES
                chunk_q = []
                seen = set()

                def enqueue_avail(tb_done):
                    avail_k = 2 * (tb_done + 1)
                    avail_q = TB * (tb_done + 1)
                    for kt in range(avail_k):
                        for s0 in range(128 * kt, S, QCH):
                            w = min(QCH, S - s0)
                            if (kt, s0) in seen or s0 + w > avail_q:
                                continue
                            seen.add((kt, s0))
                            def f(kt=kt, s0=s0):
                                if expS00[kt] is None:
                                    alloc_expS(0, 0, expS00, kt)
                                score_chunk(0, 0, expS00, kt, s0)
                            chunk_q.append(f)

                for c in chunks0[1:]:
                    c()
                enqueue_avail(0)
                for tb in range(1, NTB // 2):
                    cs = p1_chunks(tb, *p1args(True))
                    cs[0]()
                    for c in cs[1:]:
                        c()
                        for _ in range(2):
                            if chunk_q:
                                chunk_q.pop(0)()
                    enqueue_avail(tb)
                while chunk_q:
                    chunk_q.pop(0)()

                W1 = Stream([])
                for tb in range(NTB // 2, NTB):
                    W1.cl.extend(p1_chunks(tb, *p1args(False)))

                # pipeline: each PV phase hosts the next pair's first 8
                # score rows (expS slots free in exactly that order)
                expS01 = [None] * KTILES
                rows01 = make_rows(0, 1, expS01, work=W1, per_chunk=2)
                p2_pv(0, 0, expS00, pv_rows=rows01[0:8], work=W1, per_work=2)
                collective(0, 0)
                for r in rows01[8:]:
                    r()
                W1.drain()

            # p1 pools closed; open wo/p3 pools (SBUF reuse)
            with tc.tile_pool(name="wo", bufs=DT) as wo_pool, \
                 tc.tile_pool(name="p3sb", bufs=2) as p3sb, \
                 tc.tile_pool(name="rcv", bufs=2 * DT) as rcvp, \
                 tc.tile_pool(name="psW", bufs=2, space="PSUM") as psWp:
                wo_sb = []
                for dt in range(DT):
                    w = wo_pool.tile([128, D], BF16, tag="wo", name=f"wo{dt}")
                    nc.gpsimd.dma_start(w[:],
                                        woT[128 * dt:128 * (dt + 1), :])
                    wo_sb.append(w)
                A0, B0 = p3_closures(0, rcvp, p3sb, psWp, wo_sb)
                A1, B1 = p3_closures(1, rcvp, p3sb, psWp, wo_sb)

                expS10 = [None] * KTILES
                rows10 = make_rows(1, 0, expS10)
                p2_pv(0, 1, expS01, pv_rows=rows10[0:8])
                collective(0, 1)
                A0[0]()              # rcv even-dt loads (wait coll 0,0)
                for r in rows10[8:]:
                    r()

                expS11 = [None] * KTILES
                rows11 = make_rows(1, 1, expS11)
                WA = Stream(A0[1:])
                p2_pv(1, 0, expS10, pv_rows=rows11[0:8], work=WA, per_work=1)
                collective(1, 0)
                B0[0]()              # rcv odd-dt loads (wait coll 0,1)
                WA.drain()
                for r in rows11[8:]:
                    r()

                A1[0]()              # rcv even-dt b1 (wait coll 1,0)
                WB = Stream(B0[1:] + A1[1:])
                p2_pv(1, 1, expS11, work=WB, per_work=3)
                collective(1, 1)
                WB.drain()
                for c in B1:
                    c()

    nc.compile()
    return nc


def host_inputs(x, freqs_cos, freqs_sin, wq, wk, wv, wo):
    bf16 = ml_dtypes.bfloat16
    x2d = np.asarray(x, dtype=np.float32).reshape(T, D)
    # [128, NTB, DT, TB]: element [p, tb, dt, t] = x2d[TB*tb + t, 128*dt + p]
    xTt = np.ascontiguousarray(
        x2d.reshape(NTB, TB, DT, 128).transpose(3, 0, 2, 1)).astype(bf16)
    fcT = np.asarray(freqs_cos).T.astype(np.float32)   # [32, S]
    fsT = np.asarray(freqs_sin).T.astype(np.float32)
    cos4 = np.ascontiguousarray(np.tile(fcT, (4, 1))).astype(bf16)
    sin4 = np.ascontiguousarray(np.tile(fsT, (4, 1))).astype(bf16)
    woT = np.ascontiguousarray(np.asarray(wo).T).astype(bf16)
    wq = np.asarray(wq)
    wk = np.asarray(wk)
    wv = np.asarray(wv)

    permA = [h * HD + 2 * j for h in range(HPC) for j in range(HD // 2)]
    permB = [h * HD + 2 * j + 1 for h in range(HPC) for j in range(HD // 2)]
    permK = list(range(0, HD, 2)) + list(range(1, HD, 2))

    def tile_weight(w2d):
        # [out 128, D] -> [128 p, DT, 128 out]: [p, dt, e] = w2d[e, 128dt+p]
        return np.ascontiguousarray(
            w2d.T.reshape(DT, 128, 128).transpose(1, 0, 2)).astype(bf16)

    in_maps = []
    for c in range(NC_CORES):
        wq_c = wq[EQ * c: EQ * (c + 1)]
        wk_c = wk[HD * c: HD * (c + 1)]
        wv_c = wv[HD * c: HD * (c + 1)]
        wkv_c = np.concatenate([wk_c[permK], wv_c], axis=0)
        in_maps.append({
            "xTt": xTt, "cos4": cos4, "sin4": sin4,
            "wqTA": tile_weight(wq_c[permA]),
            "wqTB": tile_weight(wq_c[permB]),
            "wkvT": tile_weight(wkv_c),
            "woT": woT,
        })
    return in_maps


def host_gather(results):
    full = np.zeros((B, S, D), np.float32)
    for c in range(NC_CORES):
        o = results[c]["out"]
        for b in range(B):
            full[b, BSL * c: BSL * (c + 1), :] = o[b * BSL:(b + 1) * BSL]
    return full


_NC_CACHE = None


def _get_nc():
    global _NC_CACHE
    if _NC_CACHE is None:
        _NC_CACHE = build()
    return _NC_CACHE


def kernel(x, freqs_cos, freqs_sin, wq, wk, wv, wo):
    nc = _get_nc()
    in_maps = host_inputs(x, freqs_cos, freqs_sin, wq, wk, wv, wo)
    res = run_bass_kernel_spmd(nc, in_maps, core_ids=list(range(NC_CORES)))
    return host_gather(res.results)
